# revision 41
# baseline (speedup 1.0000x reference)
"""Trainium2 Bass kernel for zero-phase Butterworth band-stop filter (filtfilt).

Algorithm: both IIR passes of filtfilt are computed as blockwise linear algebra.
For 128-sample blocks, y_m = H0 @ u_m + W @ S_m where H0 is the Toeplitz matrix
of the filter impulse response (within-block part) and S_m stacks shifted
block-boundary data (last-8 inputs / last-8 zero-state outputs of neighboring
blocks, plus an initial-condition channel). Since all filter poles are inside
the unit circle (max radius 0.9551 -> r^128 = 2.8e-3), influence beyond J=3
blocks is below fp32 noise, so there is no sequential scan: each pass is two
full-width matmuls plus small data-stacking DMAs. Pass 2 (anticausal) uses
180-degree-rotated matrices and opposite shifts instead of flipping data.

Sharding: 32 independent lanes (batch*channel), 4 per NeuronCore across 8 cores.
"""
import os

import numpy as np

import concourse.bacc as bacc
import concourse.mybir as mybir
import concourse.tile as tile
from concourse.bass_utils import run_bass_kernel_spmd

# ---------------- problem geometry (hardcoded for this problem) ----------------
BSH, CSH, T = 4, 8, 131072      # x shape
LANES = BSH * CSH               # 32
N_CORES = 8
LPC = LANES // N_CORES          # 4 lanes per core
PADLEN = 27
BLK = 128
Z0 = 74                          # front zero padding so ext ends on block edge
NEXT = Z0 + T + 2 * PADLEN       # 131200
NB = NEXT // BLK                 # 1025 blocks per lane
COLS = LPC * NB                  # 4100 columns per core
J = 3                            # correction depth in blocks
NZ = J + 1                       # zi shifts per pass
NU = J + 1                       # ut shift groups
NR = J                           # rtilde shift groups
KS = 2 * NZ + 8 * (NU + NR)      # stack rows: 4+4+32+24 = 64
ROW_UT = 2 * NZ                  # 8
ROW_RT = ROW_UT + 8 * NU         # 40
DT = mybir.dt.float32
# float32r (4x faster PE) was tried and rejected: the correction matmul has
# ~300x internal cancellation (non-normal AR transients), and the state tails
# are produced by the main matmul, so every path needs full fp32.
DT_D = mybir.dt.float32

# blob column layout: [H0T,H0RT | U | W1T,W2T | Sinit | zeros]
BC_WR = 0                        # f32-replicated-side weights (H0T, H0RT)
BC_U = 256
BC_WF = BC_U + COLS              # correction weights (W1T, W2T)
BC_S = BC_WF + 256               # S-init region: zi rows + host-built ut rows
BC_Z = BC_S + COLS               # guaranteed-zero region for edge-zeroing DMAs
BLOB_COLS = BC_Z + 32

_matrix_cache: dict = {}
_nc_cache: dict = {}
last_exec_time_ns = None


# ---------------- host-side matrix construction (float64) ----------------
def _lfilter_zi(b, a):
    n = a.shape[0]
    A = np.zeros((n - 1, n - 1))
    A[0] = -a[1:]
    A[np.arange(1, n - 1), np.arange(0, n - 2)] = 1.0
    IminusA = np.eye(n - 1) - A.T
    Bv = b[1:] - a[1:] * b[0]
    return np.linalg.solve(IminusA, Bv)


def _build_matrices(b64, a64):
    key = (b64.tobytes(), a64.tobytes())
    if key in _matrix_cache:
        return _matrix_cache[key]
    bh = b64 / a64[0]
    ah = a64 / a64[0]
    no = 8  # filter order

    def ar_resp(drive):
        """y[t] = drive[t] - sum ah[k] y[t-k], length BLK."""
        y = np.zeros(BLK)
        for t in range(BLK):
            v = drive[t]
            for k in range(1, no + 1):
                if t - k >= 0:
                    v -= ah[k] * y[t - k]
            y[t] = v
        return y

    # h: full impulse response of B/A over one block
    drive = np.zeros(BLK)
    drive[: no + 1] = bh
    h = ar_resp(drive)
    H0 = np.zeros((BLK, BLK))
    for i in range(BLK):
        H0[i, : i + 1] = h[i::-1]

    # M: homogeneous propagator from previous 8 outputs
    M = np.zeros((BLK, 8))
    for jj in range(8):
        y = np.zeros(BLK + no)
        y[jj] = 1.0
        for t in range(no, BLK + no):
            v = 0.0
            for k in range(1, no + 1):
                v -= ah[k] * y[t - k]
            y[t] = v
        M[:, jj] = y[no:]
    P = M[BLK - 8:, :]

    # U1: contribution of previous block's last-8 inputs through the FIR part
    U1 = np.zeros((BLK, 8))
    for jj in range(8):
        dr = np.zeros(BLK)
        for t in range(BLK):
            ku = t + 8 - jj
            if 0 <= ku <= no:
                dr[t] = bh[ku]
        U1[:, jj] = ar_resp(dr)
    R1 = U1[BLK - 8:, :]

    zi = _lfilter_zi(bh, ah)

    def vzi_at(pos):
        dr = np.zeros(BLK)
        dr[pos: pos + 8] = zi
        return ar_resp(dr)

    def build_W(v_zi, zi_slot):
        """W [BLK x KS]; zi_slot 0 -> rows 0:5 (pass 1), 1 -> rows 5:10."""
        W = np.zeros((BLK, KS))
        base = NZ * zi_slot
        tz = v_zi[BLK - 8:]
        W[:, base] = v_zi
        Pj = np.eye(8)
        for j in range(J):
            W[:, base + 1 + j] = M @ Pj @ tz
            Pj = Pj @ P
        W[:, ROW_UT: ROW_UT + 8] = U1
        Pj = np.eye(8)
        for j in range(J):
            W[:, ROW_UT + 8 * (j + 1): ROW_UT + 8 * (j + 2)] = M @ Pj @ R1
            Pj = Pj @ P
        Pj = np.eye(8)
        for j in range(J):
            W[:, ROW_RT + 8 * j: ROW_RT + 8 * (j + 1)] = M @ Pj
            Pj = Pj @ P
        return W

    W1 = build_W(vzi_at(Z0), 0)
    # pass 2: rotate everything by 180 degrees (flip-free anticausal form)
    F = np.eye(BLK)[::-1]
    F8 = np.eye(8)[::-1]
    W2f = build_W(vzi_at(0), 1)
    H0R = F @ H0 @ F
    W2 = np.zeros_like(W2f)
    W2[:, :ROW_UT] = F @ W2f[:, :ROW_UT]
    for g in range(NU + NR):
        c0 = ROW_UT + 8 * g
        W2[:, c0:c0 + 8] = F @ W2f[:, c0:c0 + 8] @ F8

    out = (
        H0.T.astype(np.float32).copy(),      # lhsT for pass 1 (b)
        W1.T.astype(np.float32).copy(),      # lhsT [KS,128] pass 1 (c)
        H0R.T.astype(np.float32).copy(),
        W2.T.astype(np.float32).copy(),
    )
    _matrix_cache[key] = out
    return out


# ---------------- device kernel ----------------
def _gen_nc():
    nc = bacc.Bacc(None, target_bir_lowering=False)
    blob = nc.dram_tensor("blob", [128, BLOB_COLS], DT_D, kind="ExternalInput")
    yout = nc.dram_tensor("y", [128, COLS], DT, kind="ExternalOutput")

    HCOLS = COLS // 2               # 2050 cols per lane-half
    HLANES = LPC // 2
    HSTRIP = 410
    NHS = HCOLS // HSTRIP           # 5 strips per half

    with tile.TileContext(nc) as tc:
        with (
            tc.tile_pool(name="data", bufs=1) as data_pool,
            tc.tile_pool(name="psum", bufs=4, space="PSUM") as psum_pool,
        ):
            UW = data_pool.tile([128, 256 + COLS], DT_D, tag="UW")
            WtR = UW[:, 0:256]
            U = UW[:, 256:256 + COLS]
            WtF = data_pool.tile([KS, 256], DT, tag="WtF")
            S = data_pool.tile([KS, COLS], DT, tag="S")
            Yzs = data_pool.tile([128, COLS], DT, tag="Yzs")
            Y1 = data_pool.tile([128, COLS], DT_D, tag="Y1")
            Y2 = data_pool.tile([128, COLS], DT, tag="Y2")

            # first DMA carries both stationary weights and the first data
            # chunk (contiguous in the blob and in the UW tile)
            nc.sync.dma_start(UW[:, 0:256 + 410], blob[:, BC_WR:BC_WR + 256 + 410])
            nc.scalar.dma_start(WtF[:],
                                blob[0:KS, BC_WF:BC_WF + 256].bitcast(DT))
            for hf in range(2):
                h0 = hf * HCOLS
                h1 = h0 + HCOLS
                eng = nc.sync if hf == 0 else nc.scalar
                chunks = ((410, 1230), (1230, HCOLS)) if hf == 0 else (
                    (0, 410), (410, 1230), (1230, HCOLS))
                for o0, o1 in chunks:
                    eng.dma_start(U[:, h0 + o0:h0 + o1],
                                  blob[:, BC_U + h0 + o0:BC_U + h0 + o1])
                nc.gpsimd.dma_start(
                    S[0:ROW_RT, h0:h1],
                    blob[0:ROW_RT, BC_S + h0:BC_S + h1].bitcast(DT))

            def lv2(ap, hf):
                """[p, 2 lanes of this half, NB] view of a full-width row AP."""
                return ap.rearrange("p (l c) -> p l c", l=LPC)[
                    :, hf * HLANES:(hf + 1) * HLANES, :]

            def emit_b(pss, hf):
                Uin = U if pss == 0 else Y1
                Ht = WtR[:, 128 * pss: 128 * pss + 128]
                h0 = hf * HCOLS
                for si in range(NHS):
                    c0 = h0 + si * HSTRIP
                    c1 = c0 + HSTRIP
                    pb = psum_pool.tile([128, HSTRIP], DT, tag="pb")
                    nc.tensor.matmul(pb[:], Ht, Uin[:, c0:c1],
                                     start=True, stop=True)
                    nc.scalar.copy(Yzs[:, c0:c1], pb[:])

            def emit_stack(pss, hf):
                Uin = U if pss == 0 else Y1
                h0 = hf * HCOLS
                zsrc = blob[ROW_UT:KS,
                            BC_Z:BC_Z + HLANES * NU].bitcast(DT)
                zsrc = zsrc.rearrange("p (l c) -> p l c", l=HLANES)
                if pss == 0:
                    sv = lv2(S[ROW_RT:KS, :], hf)
                    nc.gpsimd.dma_start(sv[:, :, 0:NU],
                                        zsrc[ROW_RT - ROW_UT:])
                else:
                    sv = lv2(S[ROW_UT:KS, :], hf)
                    nc.gpsimd.dma_start(sv[:, :, NB - NU:NB], zsrc)
                # rt stack DMAs first (late-ready critical path)
                for g in range(NR):
                    sft = g + 1
                    r0 = ROW_RT + 8 * g
                    src = (lv2(Yzs[120:128, :], hf) if pss == 0
                           else lv2(Yzs[0:8, :], hf))
                    dst = lv2(S[r0:r0 + 8, :], hf)
                    eng = (nc.gpsimd, nc.sync, nc.scalar)[g % 3]
                    if pss == 0:
                        eng.dma_start(dst[:, :, sft:NB], src[:, :, 0:NB - sft])
                    else:
                        eng.dma_start(dst[:, :, 0:NB - sft], src[:, :, sft:NB])
                if pss == 1:
                    for g in range(NU):
                        sft = g + 1
                        r0 = ROW_UT + 8 * g
                        src = lv2(Uin[0:8, :].bitcast(DT), hf)
                        dst = lv2(S[r0:r0 + 8, :], hf)
                        eng = nc.sync if g % 2 == 0 else nc.scalar
                        eng.dma_start(dst[:, :, 0:NB - sft],
                                      src[:, :, sft:NB])
                    if hf == 0:
                        # zi channel for ALL lanes in one DMA per shift
                        last = NB - 1
                        span = (LPC - 1) * NB + 1
                        for sft in range(NZ):
                            nc.gpsimd.dma_start(
                                S[NZ + sft:NZ + sft + 1,
                                  last - sft:last - sft + span:NB],
                                Y1[127:128, last:last + span:NB].bitcast(DT))

            def emit_c(pss, hf):
                Yout_t = Y1 if pss == 0 else Y2
                Wc = WtF[0:KS, 128 * pss: 128 * pss + 128]
                h0 = hf * HCOLS
                for si in range(NHS):
                    c0 = h0 + si * HSTRIP
                    c1 = c0 + HSTRIP
                    pc = psum_pool.tile([128, HSTRIP], DT, tag="pc")
                    nc.tensor.matmul(pc[:], Wc, S[0:KS, c0:c1],
                                     start=True, stop=True)
                    nc.vector.tensor_add(Yout_t[:, c0:c1], Yzs[:, c0:c1],
                                         pc[:])
                if pss == 1:
                    eng = nc.sync if hf == 0 else nc.scalar
                    half = Y2[:, h0:h0 + HCOLS]
                    bounds = [(si2 * HSTRIP, min(HCOLS, (si2 + 1) * HSTRIP))
                              for si2 in range(NHS - 1)]
                    o0 = (NHS - 1) * HSTRIP
                    bounds += [(o0, o0 + HSTRIP // 2), (o0 + HSTRIP // 2, HCOLS)]
                    for o0, o1 in bounds:
                        eng.dma_start(yout[:, h0 + o0:h0 + o1],
                                      half[:, o0:o1])

            emit_b(0, 0)
            emit_stack(0, 0)
            emit_b(0, 1)
            emit_stack(0, 1)
            emit_c(0, 0)
            emit_b(1, 0)
            emit_stack(1, 0)
            emit_c(0, 1)
            emit_b(1, 1)
            emit_stack(1, 1)
            emit_c(1, 0)
            emit_c(1, 1)
    nc.compile()
    return nc


def _get_nc():
    if "nc" not in _nc_cache:
        _nc_cache["nc"] = _gen_nc()
    return _nc_cache["nc"]


# ---------------- host orchestration ----------------
def kernel(x, b=None, a=None):
    global last_exec_time_ns
    x = np.asarray(x)
    in_dtype = x.dtype
    if b is None or a is None:
        raise ValueError("need filter coefficients")
    b64 = np.asarray(b, dtype=np.float64)
    a64 = np.asarray(a, dtype=np.float64)
    H0T, W1T, H0RT, W2T = _build_matrices(b64, a64)

    xl = np.asarray(x, dtype=np.float64).reshape(LANES, T)
    left = 2 * xl[:, :1] - xl[:, PADLEN:0:-1]
    right = 2 * xl[:, -1:] - xl[:, -2:-(PADLEN + 2):-1]
    ext = np.zeros((LANES, NEXT), dtype=np.float32)
    ext[:, Z0:Z0 + PADLEN] = left
    ext[:, Z0 + PADLEN:Z0 + PADLEN + T] = xl
    ext[:, Z0 + PADLEN + T:] = right

    wblock = np.zeros((128, 512), dtype=np.float32)
    wblock[:, 0:128] = H0T
    wblock[:, 128:256] = H0RT
    wblock[0:KS, 256:384] = W1T
    wblock[0:KS, 384:512] = W2T

    in_maps = []
    for core in range(N_CORES):
        lanes = ext[core * LPC:(core + 1) * LPC]            # [LPC, NEXT]
        Ucore = lanes.reshape(LPC, NB, BLK).transpose(2, 0, 1).reshape(128, COLS)
        blob = np.zeros((128, BLOB_COLS), dtype=np.float32)
        blob[:, BC_WR:BC_WR + 256] = wblock[:, 0:256]
        blob[:, BC_U:BC_U + COLS] = Ucore
        blob[:, BC_WF:BC_WF + 256] = wblock[:, 256:512]
        # pass-1 zi rows: shifted copies of Z1 (x0 at block col 0 of each lane)
        for k in range(NZ):
            for lane in range(LPC):
                blob[k, BC_S + lane * NB + k] = lanes[lane, Z0]
        # pass-1 ut rows (shifted last-8-input rows), host-prebuilt
        ut = Ucore[120:128].reshape(8, LPC, NB)
        for g in range(NU):
            sft = g + 1
            r0 = ROW_UT + 8 * g
            for lane in range(LPC):
                c0 = BC_S + lane * NB
                blob[r0:r0 + 8, c0 + sft:c0 + NB] = ut[:, lane, 0:NB - sft]
        in_maps.append({"blob": blob})

    nc = _get_nc()
    trace = bool(int(os.environ.get("BASS_KERNEL_TRACE", "0")))
    res = run_bass_kernel_spmd(nc, in_maps, core_ids=list(range(N_CORES)),
                               trace=trace)
    last_exec_time_ns = res.exec_time_ns

    out = np.empty((LANES, T), dtype=np.float32)
    for core in range(N_CORES):
        ycore = res.results[core]["y"]                      # [128, COLS]
        lanes_y = ycore.reshape(128, LPC, NB).transpose(1, 2, 0).reshape(LPC, NEXT)
        out[core * LPC:(core + 1) * LPC] = (
            lanes_y[:, Z0 + PADLEN:Z0 + PADLEN + T])
    return out.reshape(BSH, CSH, T).astype(in_dtype)
